# revision 14
# baseline (speedup 1.0000x reference)
"""Trainium2 Bass kernel for nn_EnergyMapping (per-edge MLP -> energy sum).

Math (per molecule b):
    pre  = edge_embedding @ W1 + b1            # (E, H) with E = At*Nbr edges
    g    = softplus(pre)                        # shifted_softplus = g - log(2)
    y_e  = (g_e - log2) @ W2 + b2               # per-edge scalar
    E_b  = sum_e y_e
         = sum_h W2[h] * S[b,h] - E*log2*sum(W2) + E*b2,   S[b,h] = sum_e g[b,e,h]

Strategy: data-parallel over the batch dim (16 molecules / 8 cores = 2 each).
Each core receives its shard pre-transposed to [F=128, E=32768] so the
contraction dim F sits on SBUF partitions with perfectly contiguous DMA.
On-device per core:
  - W1 [128, 64] is the stationary operand (natural layout = lhsT).
  - Stream X^T in [128, 4096] chunks; matmul pairs of 512-edge groups into
    one PSUM tile [128, 512] via column tiling (group A -> partitions 0:64,
    group B -> 64:128), doubling TensorE throughput for the M=64 matmul.
  - One ScalarE activation per PSUM tile applies bias b1 + softplus and
    emits the per-partition row sum via accum_out into an accumulator slot.
  - Only the [128, 32] slot accumulator leaves the device; the final tiny
    dot with W2 and the b2/log2 corrections happen on host (fp64).
"""

import numpy as np

import concourse.bass as bass
import concourse.mybir as mybir
import concourse.tile as tile
from concourse import bacc
from concourse.bass_utils import run_bass_kernel_spmd

# Problem shapes (fixed by the task; kernel.py must be self-contained).
B, At, Nbr, F = 16, 256, 64, 128
H = F // 2                       # 64
N_CORES = 8
B_PER_CORE = B // N_CORES        # 2 molecules per core
EDGES_PER_MOL = At * Nbr         # 16384
E_PER_CORE = B_PER_CORE * EDGES_PER_MOL  # 32768

GROUP = 512                      # moving free dim per matmul (fp32 max, 1 PSUM bank)
PAIR = 2 * GROUP                 # edges covered per PSUM tile / ACT instruction
N_SLOTS = E_PER_CORE // PAIR     # 32 accumulator slots
SLOTS_PER_MOL = EDGES_PER_MOL // PAIR  # 16
CHUNK = 4096                     # edges per DMA chunk (2 MiB transfers)
N_CHUNKS = E_PER_CORE // CHUNK   # 8
PAIRS_PER_CHUNK = CHUNK // PAIR  # 4

LOG2 = float(np.log(2.0))

# "native": single ScalarE Softplus LUT pass — NOT supported by this
#   toolchain's act_info.json (no softplus func set) -> walrus lowering fails.
# "explog": two passes, exp then ln(1+t); both funcs live in the
#   natural_log_exp_and_others ACT table set, so no table switching.
SOFTPLUS_MODE = "explog"

_NC_CACHE = {}

# Both halves of softplus = ln(1 + exp(x)) live in this ACT table set. The
# default table-load pass picks the first set containing each function
# (exp -> exp_and_others, ln -> natural_log), which inserts a ~1.3us
# LoadActFuncSet before nearly every activation (~80us/core!). Restricting
# the candidate tables to the combined set keeps one load for the whole
# kernel. Other sets are blanked (not removed) so act_func_set_id indices
# into act_info.json stay valid.
_ACT_SET_BOTH = "natural_log_exp_and_others"


class _EnergyBacc(bacc.Bacc):
    def insert_act_table_loads(self):
        import bass_rust as _bass_rust
        from concourse.hw_specs import get_activation_tables

        has_activation = any(
            isinstance(i, mybir.InstActivation)
            for b in self.main_func.blocks
            for i in b.instructions
        )
        if not has_activation:
            return
        tables = [
            (name, funcs if name == _ACT_SET_BOTH else set())
            for name, funcs in get_activation_tables(self.m.arch).items()
        ]
        _bass_rust.insert_act_table_loads(self, tables)


def _build_nc(softplus_mode: str, reps: int = 1, loop: int = 0, parts: str = "full") -> bass.Bass:
    """Build the per-core Bass program. reps>1 repeats the whole kernel body
    unrolled; loop>0 wraps the body in a For_i hardware loop. Both are used
    only for slope-based HW timing; the output is just overwritten."""
    from contextlib import ExitStack, nullcontext

    nc = _EnergyBacc("TRN2", target_bir_lowering=False, debug=False)
    f32 = mybir.dt.float32
    xt = nc.dram_tensor("xt", [F, E_PER_CORE], f32, kind="ExternalInput")
    w1 = nc.dram_tensor("w1", [F, H], f32, kind="ExternalInput")
    b1c = nc.dram_tensor("b1c", [128, 1], f32, kind="ExternalInput")
    acc = nc.dram_tensor("acc", [128, N_SLOTS], f32, kind="ExternalOutput")

    with tile.TileContext(nc) as tc:
        with ExitStack() as ctx:
            consts = ctx.enter_context(tc.tile_pool(name="consts", bufs=1))
            xpool = ctx.enter_context(tc.tile_pool(name="xpool", bufs=3))
            psum = ctx.enter_context(tc.tile_pool(name="psum", bufs=6, space="PSUM"))
            gpool = ctx.enter_context(tc.tile_pool(name="gpool", bufs=4))
            opool = ctx.enter_context(tc.tile_pool(name="opool", bufs=1))

            w1_sb = consts.tile([F, H], f32)
            nc.sync.dma_start(w1_sb[:], w1[:, :])
            b1_sb = consts.tile([128, 1], f32)
            nc.sync.dma_start(b1_sb[:], b1c[:, :])

            acc_sb = opool.tile([128, N_SLOTS], f32)

            if loop:
                ctx.enter_context(tc.For_i(0, loop, 1))

            for _rep in range(reps):
                # Zero-init: makes overwrite-vs-accumulate accum_out semantics
                # equivalent (each slot is written by exactly one instruction).
                nc.vector.memset(acc_sb[:], 0.0)

                for c in range(N_CHUNKS):
                    xtile = xpool.tile([F, CHUNK], f32, tag="xtile")
                    nc.sync.dma_start(xtile[:], xt[:, c * CHUNK : (c + 1) * CHUNK])
                    if parts == "dma":
                        continue
                    for j in range(PAIRS_PER_CHUNK):
                        e0 = j * PAIR
                        ps = psum.tile([128, GROUP], f32, tag="ps")
                        # Column-tiled pair: two M=64 matmuls land on disjoint
                        # PSUM partition halves and run concurrently in the PE.
                        nc.tensor.matmul(
                            ps[0:64, :], w1_sb[:], xtile[:, e0 : e0 + GROUP],
                            start=True, stop=True,
                        )
                        nc.tensor.matmul(
                            ps[64:128, :], w1_sb[:], xtile[:, e0 + GROUP : e0 + PAIR],
                            start=True, stop=True,
                        )
                        slot = c * PAIRS_PER_CHUNK + j
                        g = gpool.tile([128, GROUP], f32, tag="g")
                        if parts == "dma+mm":
                            continue  # no psum consumer; PE self-serializes
                        if softplus_mode == "native":
                            nc.scalar.activation(
                                g[:], ps[:],
                                mybir.ActivationFunctionType.Softplus,
                                bias=b1_sb[:], scale=1.0,
                                accum_out=acc_sb[:, slot : slot + 1],
                            )
                        else:
                            t = gpool.tile([128, GROUP], f32, tag="t")
                            nc.scalar.activation(
                                t[:], ps[:],
                                mybir.ActivationFunctionType.Exp,
                                bias=b1_sb[:], scale=1.0,
                            )
                            nc.scalar.activation(
                                g[:], t[:],
                                mybir.ActivationFunctionType.Ln,
                                bias=1.0, scale=1.0,
                                accum_out=acc_sb[:, slot : slot + 1],
                            )

                nc.sync.dma_start(acc[:, :], acc_sb[:])
    nc.compile()
    return nc


def _get_nc() -> bass.Bass:
    if SOFTPLUS_MODE not in _NC_CACHE:
        _NC_CACHE[SOFTPLUS_MODE] = _build_nc(SOFTPLUS_MODE)
    return _NC_CACHE[SOFTPLUS_MODE]


def _make_in_maps(edge_embedding, W1, b1):
    X = np.ascontiguousarray(edge_embedding, dtype=np.float32).reshape(B, EDGES_PER_MOL, F)
    w1 = np.ascontiguousarray(W1, dtype=np.float32)
    b1c = np.concatenate([np.asarray(b1, np.float32)] * 2).reshape(128, 1)
    b1c = np.ascontiguousarray(b1c)
    in_maps = []
    for c in range(N_CORES):
        xc = X[c * B_PER_CORE : (c + 1) * B_PER_CORE].reshape(E_PER_CORE, F)
        xtc = np.ascontiguousarray(xc.T)  # [F, E] shard, F on partitions
        in_maps.append({"xt": xtc, "w1": w1, "b1c": b1c})
    return in_maps


def _finalize(results, W1, b1, W2, b2):
    W2v = np.asarray(W2, np.float64).reshape(H)
    b2v = float(np.asarray(b2).reshape(()))
    out = np.empty((B, 1), np.float32)
    corr = -EDGES_PER_MOL * LOG2 * float(W2v.sum()) + EDGES_PER_MOL * b2v
    for c in range(N_CORES):
        acc = np.asarray(results[c]["acc"], np.float64)  # [128, N_SLOTS]
        S = acc[0:64, :] + acc[64:128, :]  # per-h, per-slot softplus sums
        for i in range(B_PER_CORE):
            b = c * B_PER_CORE + i
            Sg = S[:, i * SLOTS_PER_MOL : (i + 1) * SLOTS_PER_MOL].sum(axis=1)
            out[b, 0] = np.float32(Sg @ W2v + corr)
    return out


def kernel_with_results(edge_embedding, W1, b1, W2, b2, trace=False, **run_kwargs):
    nc = _get_nc()
    in_maps = _make_in_maps(edge_embedding, W1, b1)
    br = run_bass_kernel_spmd(nc, in_maps, list(range(N_CORES)), trace=trace, **run_kwargs)
    out = _finalize(br.results, W1, b1, W2, b2)
    return out, br


def kernel(edge_embedding, W1, b1, W2, b2):
    out, _ = kernel_with_results(edge_embedding, W1, b1, W2, b2)
    return out
